# revision 1
# baseline (speedup 1.0000x reference)
"""Trainium2 Bass kernel for local sparse attention (k=16 neighbors).

Reference computation (b=4, n=8192, k=16, d=128):
    Q = src @ Wq.T ; K = tgt @ Wk.T ; V = tgt @ Wv.T
    scores = einsum('bnkd,bnd->bnk', K, Q) / sqrt(d)
    out = einsum('bnk,bnkd->bnd', softmax(scores), V)

Algebraic restructuring (key to reaching the memory roofline):
    scores[n,k] = tgt[n,k,:] . (src[n,:] @ (Wq.T @ Wk) / sqrt(d))
    out[n,:]    = (sum_k attn[n,k] * tgt[n,k,:]) @ Wv.T
so the 34-GFLOP K/V projections are never materialized; tgt streams from
HBM exactly once in its natural layout (8KB contiguous per point).

Per 128-point tile, both heavy steps run as ONE fused DVE pass each via a
custom multiply-cumsum op (out = cumsum(in0*in1) along the free dim);
per-neighbor segment sums are recovered as differences of the cumulative
sum at segment ends.

Sharding: data-parallel over flattened (b*n) = 32768 points across 8
NeuronCores; attention is fully local per point.
"""

import math

import numpy as np

# ---- problem constants (hardcoded per the contract) ----
B, N, KNBR, D = 4, 8192, 16, 128
NCORES = 8
PTS_TOTAL = B * N               # 32768
PTS_CORE = PTS_TOTAL // NCORES  # 4096
TILE_PTS = 128
_cached = {}


def _register_mul_cumsum():
    """Register the custom DVE op out[p,t] = cumsum_t(in0[p,t]*in1[p,t])."""
    import concourse.dve_ops as dve_ops
    for op in dve_ops.OPS:
        if op.name == "MUL_CUMSUM_ANT":
            return op
    from concourse.dve_spec import Spec, Src0, Src1, AluOp, scan, lower
    from concourse.dve_uop import DveOpSpec
    from concourse.dve_table_gen import dve_ver_for
    from concourse.dve_ops import DveOp, _CUSTOM_DVE_ROW_BASE

    spec = Spec(
        body=scan(AluOp.ADD, Src0 * Src1),
        reference=lambda in0, in1, s0, s1, imm2: np.cumsum(
            in0.reshape(in0.shape[0], -1).astype(np.float32)
            * in1.reshape(in0.shape[0], -1).astype(np.float32), axis=1),
    )
    ver = dve_ver_for("TRN2")
    row = _CUSTOM_DVE_ROW_BASE + len(dve_ops.OPS)
    sha = DveOpSpec(name="MUL_CUMSUM_ANT", opcode=row,
                    uops=lower(spec, ver=ver), rd1_en=True).sha(ver)
    op = DveOp("MUL_CUMSUM_ANT", spec, subdim=False, uops_sha={ver: sha})
    dve_ops.OPS.append(op)
    dve_ops._SUB_OPCODE_FOR_NAME[op.name] = row
    dve_ops.CUSTOM_DVE_SPECS[op.name] = spec
    return op


def _build_program(pts_core=PTS_CORE, num_devices=NCORES):
    import concourse.bacc as bacc
    import concourse.bass as bass
    import concourse.tile as tile
    from concourse import mybir

    mcs = _register_mul_cumsum()
    ntiles = pts_core // TILE_PTS

    nc = bacc.Bacc("TRN2", target_bir_lowering=False, debug=False,
                   num_devices=num_devices)

    f32 = mybir.dt.float32
    src_h = nc.dram_tensor("src_sh", [pts_core, D], f32, kind="ExternalInput").ap()
    tgt_h = nc.dram_tensor("tgt_sh", [pts_core * KNBR, D], f32, kind="ExternalInput").ap()
    wqk_h = nc.dram_tensor("wqk", [D, D], f32, kind="ExternalInput").ap()
    wvt_h = nc.dram_tensor("wvt", [D, D], f32, kind="ExternalInput").ap()
    iden_h = nc.dram_tensor("iden", [D, D], f32, kind="ExternalInput").ap()
    out_h = nc.dram_tensor("out_sh", [pts_core, D], f32, kind="ExternalOutput").ap()

    ALU = mybir.AluOpType
    ACTF = mybir.ActivationFunctionType

    with tile.TileContext(nc) as tc:
        with (
            tc.tile_pool(name="consts", bufs=1) as consts,
            tc.tile_pool(name="srcp", bufs=1) as srcp,
            tc.tile_pool(name="qwp", bufs=ntiles) as qwp,
            tc.tile_pool(name="tnp", bufs=3) as tnp,
            tc.tile_pool(name="big", bufs=2) as big,
            tc.tile_pool(name="small", bufs=4) as small,
            tc.tile_pool(name="ps", bufs=4, space="PSUM") as ps,
        ):
            tgt_v = tgt_h.rearrange("(n k) d -> n k d", k=KNBR)
            src_v = src_h.rearrange("(t p) d -> p t d", p=TILE_PTS)

            # the very first DMA triggers on the (serial) Sync queue are the
            # first pairs' tgt tiles — everything downstream waits on them
            npairs = ntiles // 2
            tn_tiles = {}
            tnp_ref = tnp

            def load_tn(tp):
                p0 = tp * 2 * TILE_PTS
                tn = tnp_ref.tile([TILE_PTS, 2, KNBR, D], f32, tag="tn")
                nc.sync.dma_start(out=tn[:, 0], in_=tgt_v[p0:p0 + TILE_PTS])
                nc.sync.dma_start(out=tn[:, 1],
                                  in_=tgt_v[p0 + TILE_PTS:p0 + 2 * TILE_PTS])
                tn_tiles[tp] = tn

            for tp in range(min(2, npairs)):
                load_tn(tp)

            wqk_sb = consts.tile([D, D], f32)
            nc.sync.dma_start(out=wqk_sb, in_=wqk_h)
            wvt_sb = consts.tile([D, D], f32)
            nc.sync.dma_start(out=wvt_sb, in_=wvt_h)
            iden_sb = consts.tile([D, D], f32)
            nc.sync.dma_start(out=iden_sb, in_=iden_h)

            # queries: Qw[t] = src_tile[t] @ Wqk (Wqk includes the 1/sqrt(d)
            # scale); emitted with a small lookahead so the ACT copies
            # interleave with the main loop's exps instead of queueing ahead
            # of them (engines dispatch roughly in program order).
            src_all = srcp.tile([TILE_PTS, ntiles, D], f32)
            for c in range(0, ntiles, 4):
                ce = min(c + 4, ntiles)
                nc.sync.dma_start(out=src_all[:, c:ce, :], in_=src_v[:, c:ce, :])
            qw_tiles = {}

            def emit_qw(t):
                st_ps = ps.tile([D, TILE_PTS], f32, tag="pss")
                nc.tensor.transpose(st_ps, src_all[:, t, :], iden_sb)
                st_sb = small.tile([D, TILE_PTS], f32, tag="st")
                nc.scalar.copy(st_sb, st_ps)
                qw_ps = ps.tile([TILE_PTS, D], f32, tag="pss")
                nc.tensor.matmul(qw_ps, lhsT=st_sb, rhs=wqk_sb, start=True, stop=True)
                qw_sb = qwp.tile([TILE_PTS, D], f32, tag="qw")
                nc.scalar.copy(qw_sb, qw_ps)
                qw_tiles[t] = qw_sb

            # main loop processes PAIRS of 128-pt tiles so the small DVE ops
            # (segment diffs, reciprocal) amortize their fixed overhead
            LOOKAHEAD = 3  # pairs
            for t in range(min(2 * LOOKAHEAD, ntiles)):
                emit_qw(t)
            CW = 1 + KNBR * D          # guarded cumsum width per half
            for tp in range(npairs):
                for t in (2 * (tp + LOOKAHEAD), 2 * (tp + LOOKAHEAD) + 1):
                    if t < ntiles:
                        emit_qw(t)
                if tp + 2 < npairs:
                    load_tn(tp + 2)
                p0 = tp * 2 * TILE_PTS
                tn = tn_tiles.pop(tp)

                # ---- scores: one fused multiply-cumsum over (k,d) per half;
                # a zeroed guard column at flat offset 0 makes the
                # segment-difference a single tensor_tensor subtract.
                cum1 = big.tile([TILE_PTS, 2, CW], f32, tag="cum1")
                nc.gpsimd.memset(cum1[:, :, 0:1], 0.0)
                for h in range(2):
                    qw_sb = qw_tiles[2 * tp + h]
                    qw_bk = bass.AP(tensor=qw_sb.tensor, offset=qw_sb.offset,
                                    ap=[qw_sb.ap[0], [0, KNBR], [1, D]])
                    nc.vector._custom_dve(mcs, out=cum1[:, h, 1:],
                                          in0=tn[:, h], in1=qw_bk)
                # segment ends at flat offsets {0, 128, ..., 2048} per half
                ends1_hi = bass.AP(tensor=cum1.tensor, offset=cum1.offset + D,
                                   ap=[cum1.ap[0], [CW, 2], [D, KNBR]])
                ends1_lo = bass.AP(tensor=cum1.tensor, offset=cum1.offset,
                                   ap=[cum1.ap[0], [CW, 2], [D, KNBR]])
                scores = small.tile([TILE_PTS, 2, KNBR], f32, tag="sc")
                nc.vector.tensor_sub(scores, ends1_hi, ends1_lo)

                # ---- softmax over k (scores bounded; skip max-subtraction);
                # exp's accum_out gives the denominator in the same op
                e_sb = small.tile([TILE_PTS, 2, KNBR], f32, tag="e")
                den = small.tile([TILE_PTS, 2], f32, tag="den")
                for h in range(2):
                    nc.scalar.activation(e_sb[:, h], scores[:, h], ACTF.Exp,
                                         accum_out=den[:, h:h + 1])
                rden = small.tile([TILE_PTS, 2], f32, tag="rden")
                nc.vector.reciprocal(rden, den)

                # ---- ctx: fused multiply-cumsum over (d,k) per half: tn read
                # d-outer/k-inner; E broadcast over d
                cum2 = big.tile([TILE_PTS, 2, CW], f32, tag="cum2")
                nc.gpsimd.memset(cum2[:, :, 0:1], 0.0)
                for h in range(2):
                    tn_dk = bass.AP(tensor=tn.tensor,
                                    offset=tn.offset + h * KNBR * D,
                                    ap=[tn.ap[0], [1, D], [D, KNBR]])
                    e_bd = bass.AP(tensor=e_sb.tensor,
                                   offset=e_sb.offset + h * KNBR,
                                   ap=[e_sb.ap[0], [0, D], [1, KNBR]])
                    nc.vector._custom_dve(mcs, out=cum2[:, h, 1:],
                                          in0=e_bd, in1=tn_dk)
                # segment ends at flat offsets {0, 16, ..., 2048} per half
                ends2_hi = bass.AP(tensor=cum2.tensor, offset=cum2.offset + KNBR,
                                   ap=[cum2.ap[0], [CW, 2], [KNBR, D]])
                ends2_lo = bass.AP(tensor=cum2.tensor, offset=cum2.offset,
                                   ap=[cum2.ap[0], [CW, 2], [KNBR, D]])
                ctx = small.tile([TILE_PTS, 2, D], f32, tag="ctx")
                nc.vector.tensor_sub(ctx, ends2_hi, ends2_lo)

                # ---- out = (ctx/den) @ Wv.T, per half
                for h in range(2):
                    ctxt_ps = ps.tile([D, TILE_PTS], f32, tag="pss")
                    nc.tensor.transpose(ctxt_ps, ctx[:, h], iden_sb)
                    ctxt_sb = small.tile([D, TILE_PTS], f32, tag="ctxt")
                    nc.scalar.copy(ctxt_sb, ctxt_ps)
                    out_ps = ps.tile([TILE_PTS, D], f32, tag="pss")
                    nc.tensor.matmul(out_ps, lhsT=ctxt_sb, rhs=wvt_sb,
                                     start=True, stop=True)
                    out_sb = small.tile([TILE_PTS, D], f32, tag="outsb")
                    nc.scalar.activation(out_sb, out_ps, ACTF.Copy,
                                         scale=rden[:, h:h + 1])
                    q0 = p0 + h * TILE_PTS
                    nc.sync.dma_start(out=out_h[q0:q0 + TILE_PTS], in_=out_sb)

    nc.compile()
    return nc


def kernel(src, tgt, Wq, Wk, Wv):
    from concourse.bass_utils import run_bass_kernel_spmd

    src = np.ascontiguousarray(src, dtype=np.float32)
    tgt = np.ascontiguousarray(tgt, dtype=np.float32)

    scale = 1.0 / math.sqrt(D)
    wqk = (Wq.astype(np.float64).T @ Wk.astype(np.float64) * scale).astype(np.float32)
    wvt = np.ascontiguousarray(Wv.astype(np.float32).T)
    iden = np.eye(D, dtype=np.float32)

    src_f = src.reshape(PTS_TOTAL, D)
    tgt_f = tgt.reshape(PTS_TOTAL * KNBR, D)

    if "nc" not in _cached:
        _cached["nc"] = _build_program()
    nc = _cached["nc"]

    in_maps = []
    for c in range(NCORES):
        p0, p1 = c * PTS_CORE, (c + 1) * PTS_CORE
        in_maps.append({
            "src_sh": np.ascontiguousarray(src_f[p0:p1]),
            "tgt_sh": np.ascontiguousarray(tgt_f[p0 * KNBR:p1 * KNBR]),
            "wqk": wqk,
            "wvt": wvt,
            "iden": iden,
        })

    _cached["in_maps"] = in_maps
    res = run_bass_kernel_spmd(nc, in_maps, core_ids=list(range(NCORES)))
    out = np.concatenate([r["out_sh"] for r in res.results], axis=0)
    return out.reshape(B, N, D).astype(np.float32)


def __getattr__(name):
    if name == "_last_in_maps":
        return _cached.get("in_maps")
    raise AttributeError(name)

